# revision 5
# baseline (speedup 1.0000x reference)
"""Trainium2 kernel: segment-mean aggregation (nn_Aggregate).

Computes, for S = batch_size * n_nodes segments:
    out[s // N, s % N, :] = mean of edge_vec rows whose selected_edges[:,5] == s

Strategy (8 NeuronCores, SPMD, no collectives):
  * Host shards edges by DESTINATION segment: segments are assigned to cores
    (contiguous 40K ranges) and, within a core, bin-packed into NB blocks of
    <=128 segments and <= K*128 edges.  Edges are permuted (host-side, pure
    index metadata + one gather) into a padded per-block stream.
  * Device, per block: build a one-hot matrix onehot[e, s] = (seg_rel[e] == s)
    with one DVE is_equal op, then accumulate sums[s, d] += onehot.T @ vec
    with K matmuls into PSUM.  Epilogue multiplies by 1/count (per-partition
    scale on the Scalar engine) and DMAs the block's 128 output rows out.
  * Host inverse-permutes the per-core output rows into the [B, N, D] grid.

All floating-point reduction work (the ~1 GB of edge summation) happens on
device; the host only computes integer index metadata (bincount/argsort) and
performs the shard permutation implied by the sharding strategy.
"""

import os
import sys

import numpy as np

for _p in ("/opt/trn_rl_repo", "/root/.axon_site/_ro/trn_rl_repo"):
    if os.path.isdir(_p) and _p not in sys.path:
        sys.path.append(_p)

# Problem constants (hardcoded per spec nn_Aggregate_8985071583847)
E = 2_000_000
D = 128
B = 16
N = 20_000
S = B * N
NCORES = 8

# Kernel tiling parameters
G = 128      # segment slots per block (= PSUM partition count)
K = 6        # edge tiles (of 128) per block
SB = 4       # blocks per super-block (DMA batching granularity)
PAD_SEGREL = 999.0  # exact in fp16; never matches iota in [0, 128)


def _pack_segments(counts2, NB, cap):
    """Bin-pack each core's segments into NB blocks.

    counts2: [ncores, seg_per_core] per-segment edge counts.
    Returns (ok, binid, slocal, estart) each [ncores, seg_per_core]:
      binid  - block index within the core
      slocal - segment's slot within the block (PSUM partition row)
      estart - first edge slot of this segment within its block
    Packing: sort by count descending, deal round-robin across NB blocks.
    """
    ncores, spc = counts2.shape
    if spc > NB * G:
        return False, None, None, None
    binid = np.empty((ncores, spc), np.int32)
    slocal = np.empty((ncores, spc), np.int32)
    estart = np.empty((ncores, spc), np.int64)
    ranks = np.arange(spc)
    rb = (ranks % NB).astype(np.int32)
    rs = (ranks // NB).astype(np.int32)
    nrows = -(-spc // NB)
    for c in range(ncores):
        cc = counts2[c]
        order = np.argsort(-cc, kind="stable")
        binid[c][order] = rb
        slocal[c][order] = rs
        arr = np.zeros(nrows * NB, dtype=np.int64)
        arr[:spc] = cc[order]
        arr = arr.reshape(nrows, NB)
        if arr.sum(0).max() > cap:
            return False, None, None, None
        excl = np.cumsum(arr, axis=0) - arr  # edges before this rank in its bin
        estart[c][order] = excl.ravel()[:spc]
    return True, binid, slocal, estart


def _prepare(edge_vec, seg, s_total, ncores, k, sb):
    """Host-side sharding: returns per-core input arrays + unshard map."""
    e_total, d = edge_vec.shape
    spc = s_total // ncores
    cap = k * G

    counts = np.bincount(seg, minlength=s_total).astype(np.int64)
    counts2 = counts.reshape(ncores, spc)

    nb = max(
        int(np.ceil(counts2.sum(1).max() / cap)),
        int(np.ceil(spc / G)),
    )
    nb = -(-nb // sb) * sb
    while True:
        ok, binid, slocal, estart = _pack_segments(counts2, nb, cap)
        if ok:
            break
        nb += sb

    binid = binid.ravel()
    slocal = slocal.ravel()
    estart = estart.ravel()
    core_s = np.arange(s_total, dtype=np.int64) // spc

    # Edges grouped by segment (stable sort keeps determinism)
    order_e = np.argsort(seg, kind="stable")
    seg_sorted = seg[order_e]
    seg_start = np.zeros(s_total + 1, np.int64)
    np.cumsum(counts, out=seg_start[1:])
    within = np.arange(e_total, dtype=np.int64) - seg_start[seg_sorted]

    slot = estart[seg_sorted] + within          # edge slot within block [0, cap)
    t_e = slot // 128
    p_e = slot % 128
    b_e = binid[seg_sorted].astype(np.int64)
    c_e = core_s[seg_sorted]

    nbrows = nb * k * 128                        # stream rows per core
    row = ((b_e // sb) * 128 + p_e) * (sb * k) + (b_e % sb) * k + t_e
    grow = c_e * nbrows + row

    vec16 = np.ascontiguousarray(edge_vec, dtype=np.float16)
    stream = np.zeros((ncores * nbrows, d), np.float16)
    stream[grow] = vec16[order_e]

    # seg_rel table: [ncores, 128(part), nb*k], value = slot row of the edge's
    # segment inside its block; PAD for unused edge slots.
    segrel = np.full(ncores * 128 * nb * k, PAD_SEGREL, np.float16)
    segrel[(c_e * 128 + p_e) * (nb * k) + b_e * k + t_e] = slocal[seg_sorted]
    segrel = segrel.reshape(ncores, 128, nb * k)

    # 1/count per (core, slot row, block); 1.0 for empty slots.
    invc = np.ones(ncores * 128 * nb, np.float32)
    invc[(core_s * 128 + slocal) * nb + binid] = 1.0 / np.maximum(counts, 1)
    invc = invc.reshape(ncores, 128, nb)

    iota = np.broadcast_to(
        np.arange(128, dtype=np.float16), (128, 128)
    ).copy()

    # Inverse map: device out row (core, b*128 + slocal) -> global segment id
    seg_of = np.full(ncores * nb * 128, -1, np.int64)
    seg_of[core_s * (nb * 128) + binid * 128 + slocal] = np.arange(s_total)
    seg_of = seg_of.reshape(ncores, nb * 128)

    return nb, stream, segrel, invc, iota, seg_of


def _build_graph(nb, k, sb, d):
    import concourse.tile as tile
    from concourse import bacc, mybir

    f16 = mybir.dt.float16
    f32 = mybir.dt.float32
    nsb = nb // sb
    sbk = sb * k

    nc = bacc.Bacc()
    vec_p = nc.declare_dram_parameter("vec", [nsb * 128, sbk * 128], f16, isOutput=False)
    srel_p = nc.declare_dram_parameter("srel", [128, nb * k], f16, isOutput=False)
    invc_p = nc.declare_dram_parameter("invc", [128, nb], f32, isOutput=False)
    iota_p = nc.declare_dram_parameter("iota", [128, 128], f16, isOutput=False)
    out_p = nc.declare_dram_parameter("out", [nb * 128, d], f32, isOutput=True)

    with tile.TileContext(nc) as tc:
        with tc.tile_pool(name="const", bufs=1) as cpool, \
             tc.tile_pool(name="vecp", bufs=3) as vpool, \
             tc.tile_pool(name="ohp", bufs=3) as opool, \
             tc.tile_pool(name="resp", bufs=3) as rpool, \
             tc.tile_pool(name="psp", bufs=4, space="PSUM") as ppool:

            srel_t = cpool.tile([128, nb * k], f16)
            nc.sync.dma_start(out=srel_t[:], in_=srel_p[:, :])
            invc_t = cpool.tile([128, nb], f32)
            nc.sync.dma_start(out=invc_t[:], in_=invc_p[:, :])
            iota_t = cpool.tile([128, 128], f16)
            nc.sync.dma_start(out=iota_t[:], in_=iota_p[:, :])

            for isb in range(nsb):
                vt = vpool.tile([128, sbk * 128], f16)
                nc.sync.dma_start(
                    out=vt[:], in_=vec_p[isb * 128 : (isb + 1) * 128, :]
                )
                oh = opool.tile([128, sbk * 128], f16)
                nc.vector.tensor_tensor(
                    out=oh[:].rearrange("p (a s) -> p a s", s=128),
                    in0=srel_t[:, isb * sbk : (isb + 1) * sbk]
                    .unsqueeze(2)
                    .to_broadcast([128, sbk, 128]),
                    in1=iota_t[:].unsqueeze(1).to_broadcast([128, sbk, 128]),
                    op=mybir.AluOpType.is_equal,
                )
                ps = ppool.tile([128, sb * 128], f32, space="PSUM")
                for j in range(sb):
                    for t in range(k):
                        col = (j * k + t) * 128
                        nc.tensor.matmul(
                            out=ps[:, j * 128 : (j + 1) * 128],
                            lhsT=oh[:, col : col + 128],
                            rhs=vt[:, col : col + 128],
                            start=(t == 0),
                            stop=(t == k - 1),
                        )
                ot = rpool.tile([128, sb * d], f32)
                for j in range(sb):
                    nc.scalar.mul(
                        out=ot[:, j * d : (j + 1) * d],
                        in_=ps[:, j * 128 : (j + 1) * 128],
                        mul=invc_t[:, isb * sb + j : isb * sb + j + 1],
                    )
                nc.scalar.dma_start(
                    out=out_p[isb * sb * 128 : (isb + 1) * sb * 128, :].rearrange(
                        "(a p) d -> p a d", p=128
                    ),
                    in_=ot[:].rearrange("p (a d) -> p a d", d=d),
                )
    nc.compile()
    return nc


def _run(edge_vec, seg, s_total=S, ncores=NCORES, k=K, sb=SB, trace=False):
    from concourse.bass_utils import run_bass_kernel_spmd

    edge_vec = np.asarray(edge_vec, dtype=np.float32)
    seg = np.asarray(seg, dtype=np.int64)
    d = edge_vec.shape[1]

    nb, stream, segrel, invc, iota, seg_of = _prepare(
        edge_vec, seg, s_total, ncores, k, sb
    )
    nc = _build_graph(nb, k, sb, d)

    nbrows = nb * k * 128
    in_maps = [
        {
            "vec": stream[c * nbrows : (c + 1) * nbrows].reshape(
                (nb // sb) * 128, sb * k * 128
            ),
            "srel": segrel[c],
            "invc": invc[c],
            "iota": iota,
        }
        for c in range(ncores)
    ]
    res = run_bass_kernel_spmd(
        nc, in_maps, core_ids=list(range(ncores)), trace=trace
    )

    dev = np.concatenate([res.results[c]["out"] for c in range(ncores)], axis=0)
    out_flat = np.zeros((s_total, d), np.float32)
    mask = seg_of.ravel() >= 0
    out_flat[seg_of.ravel()[mask]] = dev[mask]
    return out_flat, res, nc


def kernel(edge_vec, selected_edges, num_segments=S, batch_size=B, n_nodes=N):
    selected_edges = np.asarray(selected_edges)
    seg = np.asarray(selected_edges[:, 5], dtype=np.int64)
    s_total = int(num_segments)
    out_flat, _, _ = _run(edge_vec, seg, s_total=s_total)
    return out_flat.reshape(int(batch_size), int(n_nodes), -1)
